# revision 1
# baseline (speedup 1.0000x reference)
"""Trainium2 Bass kernel for nn_ADTNLinear (3-layer pairwise tensor-network).

Strategy: data-parallel over batch (8 cores x 512 rows). Host folds the rank
axes of the gates into per-pair 64x16 weight blocks, applies the input
permutation, and precomputes layer-0's outer-product operand slabs. On
device, each layer processes 32 groups of 8 site-pairs:
  - operand slabs (x1 replicated over j / x2 tiled over i, [128, 2048] bf16)
    are DMA-gathered from an HBM mirror of the layer input (layer 0: loaded
    directly from the host-built slabs),
  - one DVE tensor_mul forms z = x1rep * x2rep,
  - 16 tile_position-packed 32x32 TensorE matmuls contract z against the
    folded weights into PSUM ([128, 512] f32),
  - ScalarE copies PSUM to an SBUF bounce (bf16) which DMAs to the next
    layer's HBM mirror; layer 2 fuses the alpha*scale+bias affine and DMAs
    the f32 result out.
Layer 1 (offset=1, 255 pairs) writes a shifted "s-layout" mirror so PSUM
writebacks stay partition-aligned; layer 2's gather offsets absorb the shift.
"""

import sys

sys.path.insert(0, "/opt/trn_rl_repo")

import numpy as np

import concourse.bacc as bacc
import concourse.bass as bass
import concourse.mybir as mybir
from concourse import tile
from concourse.bass import _add_dep_helper
from concourse.bass_utils import run_bass_kernel_spmd


def _minst(ret):
    return ret.ins if hasattr(ret, "ins") else ret


def _dep(after, before, reason):
    """after must run after before (raw-AP reads are invisible to Tile)."""
    if before is None:
        return
    _add_dep_helper(_minst(after), _minst(before), sync=True, reason=reason)

NCORES = 8
BATCH = 4096
BL = BATCH // NCORES  # 512 batch rows per core
NS = 512  # sites
D = 8  # features per site
F = NS * D  # 4096
NBLK = F // 128  # 32 feature blocks
BF16 = mybir.dt.bfloat16
F32 = mybir.dt.float32


def _fold_gates(g):
    """gates (P, rl, rr, i, j, o, p) -> W (P, 64, 16) with z-row = 8*i + j."""
    G = g.sum(axis=(1, 2))  # (P, i, j, o, p)
    w1 = G.sum(-1)  # (P, i, j, m)  site1 outputs
    w2 = G.sum(-2)  # (P, i, j, n)  site2 outputs
    P = G.shape[0]
    W = np.concatenate([w1, w2], axis=-1)  # (P, i, j, 16)
    return W.reshape(P, 64, 16).astype(np.float32)


def _layer_pairs(layer):
    return 256 if layer != 1 else 255


def _wbuf_for_layer(W, layer):
    """Host-side SBUF image of the packed weights: (128, 32*128) f32.

    Tile (r, c) of group g holds lhsT[32, 32]: rows = z-rows 32r..32r+32 of
    pair kA=8g+2c (r<2, cols 0:16) or pair kB=8g+2c+1 (r>=2, cols 16:32).
    """
    P = W.shape[0]
    wb = np.zeros((128, NBLK * 128), np.float32)
    for g in range(32):
        for c in range(4):
            kA = 8 * g + 2 * c
            kB = kA + 1
            col = 128 * g + 32 * c
            if kA < P:
                wb[0:64, col : col + 16] = W[kA]
            if kB < P:
                wb[64:128, col + 16 : col + 32] = W[kB]
    return wb


def _src_feat_row(layer, g, c, a, operand):
    """Feature row (0..4095) in the layer's input mirror for one gather.

    The mirror for layer l stores feature f at row: f (canonical, layers
    0/1) or (f-8) % 4096 reordered per the s-layout (layer 2): feats
    8..4088 at rows 0..4080, site0 at rows 3952+? -- concretely the
    s-layout mirror row of feat f is (f-8) mod 128 + 128*((f-8)//128) for
    f in [8, 4088), site0 (f 0..8) at block31 parts 112..120 = rows
    4080..4088, site511 (f 4088..96) at rows 4088..4096.
    """
    o = 1 if layer == 1 else 0
    k = 8 * g + 2 * c + a
    s = 2 * k + o + (1 if operand == 2 else 0)
    f = 8 * s
    if layer in (0, 1):
        return f
    if s == 0:
        return 31 * 128 + 112
    if s == 511:
        return 31 * 128 + 120
    return f - 8


def build_bass(bl=BL):
    """Build the single-core graph (SPMD: all 8 cores run it on their shard)."""
    nc = bacc.Bacc(None, target_bir_lowering=False, debug=False)

    z1l0 = nc.declare_dram_parameter("z1l0", [32, 128, 4 * bl], BF16, isOutput=False)
    z2l0 = nc.declare_dram_parameter("z2l0", [32, 128, 4 * bl], BF16, isOutput=False)
    wbs = [
        nc.declare_dram_parameter(f"w{l}", [128, NBLK * 128], BF16, isOutput=False)
        for l in range(3)
    ]
    scb_d = nc.declare_dram_parameter("scb", [128, NBLK], F32, isOutput=False)
    bib_d = nc.declare_dram_parameter("bib", [128, NBLK], F32, isOutput=False)
    out_d = nc.declare_dram_parameter("out", [NBLK, 128, bl], F32, isOutput=True)

    with tile.TileContext(nc) as tc:
        with (
            tc.tile_pool(name="wbuf", bufs=1) as wpool,
            tc.tile_pool(name="zbuf", bufs=4) as zpool,
            tc.tile_pool(name="obuf", bufs=4) as opool,
            tc.tile_pool(name="dram", bufs=1, space="DRAM") as dpool,
            tc.tile_pool(name="psum", bufs=6, space="PSUM") as pspool,
        ):
            wt = [
                wpool.tile([128, NBLK * 128], BF16, tag=f"w{l}", name=f"wt{l}")
                for l in range(3)
            ]
            scb = wpool.tile([128, NBLK], F32, tag="scb")
            bib = wpool.tile([128, NBLK], F32, tag="bib")
            hm1 = dpool.tile([F, bl], BF16, tag="hm1")  # canonical mirror
            hm2 = dpool.tile([F, bl], BF16, tag="hm2")  # s-layout mirror

            for l in range(3):
                nc.sync.dma_start(out=wt[l][:], in_=wbs[l][:])
            nc.sync.dma_start(out=scb[:], in_=scb_d[:])
            nc.sync.dma_start(out=bib[:], in_=bib_d[:])

            mirrors = {1: hm1, 2: hm2}
            # writers[(mirror_idx, blk)] = insts that wrote that 128-row block
            writers = {}

            for layer in range(3):
                P = _layer_pairs(layer)
                for g in range(32):
                    npair = min(8, P - 8 * g)  # 8, or 7 for layer-1 group 31
                    zb1 = zpool.tile([128, 4 * bl], BF16, tag="zb1")
                    zb2 = zpool.tile([128, 4 * bl], BF16, tag="zb2")
                    zp = zpool.tile([128, 4 * bl], BF16, tag="zp")
                    if layer == 0:
                        nc.sync.dma_start(out=zb1[:], in_=z1l0[g])
                        nc.scalar.dma_start(out=zb2[:], in_=z2l0[g])
                    else:
                        hm = mirrors[layer]
                        if npair < 8:  # partial group: zero unwritten region
                            nc.vector.memset(zb1[:], 0.0)
                            nc.vector.memset(zb2[:], 0.0)
                        for c in range(4):
                            for a in range(2):
                                if 8 * g + 2 * c + a >= P:
                                    continue
                                f1 = _src_feat_row(layer, g, c, a, 1)
                                # x1 replicated over j: iter (i, j-rep, b)
                                ap1 = bass.AP(
                                    hm[:].tensor,
                                    f1 * bl,
                                    [[bl, 8], [0, 8], [1, bl]],
                                )
                                d1 = nc.sync.dma_start(
                                    out=zb1[
                                        64 * a : 64 * a + 64,
                                        c * bl : (c + 1) * bl,
                                    ],
                                    in_=ap1,
                                )
                                for w in writers.get((layer, f1 // 128), ()):
                                    _dep(d1, w, "x1 gather after mirror write")
                                f2 = _src_feat_row(layer, g, c, a, 2)
                                # x2 tiled over i: iter (i-rep, j, b)
                                ap2 = bass.AP(
                                    hm[:].tensor,
                                    f2 * bl,
                                    [[0, 8], [bl, 8], [1, bl]],
                                )
                                d2 = nc.scalar.dma_start(
                                    out=zb2[
                                        64 * a : 64 * a + 64,
                                        c * bl : (c + 1) * bl,
                                    ],
                                    in_=ap2,
                                )
                                for w in writers.get((layer, f2 // 128), ()):
                                    _dep(d2, w, "x2 gather after mirror write")
                    nc.vector.tensor_mul(zp[:], zb1[:], zb2[:])

                    psum = pspool.tile([128, bl], F32, tag="ps")
                    for c in range(4):
                        if 8 * g + 2 * c >= P:
                            continue
                        # block-diag 2-pair weights: K=128 (2x64 z-rows), M=32;
                        # explicit col-group tile_position -> 4 concurrent MMs
                        nc.tensor.matmul(
                            psum[32 * c : 32 * c + 32, :],
                            lhsT=wt[layer][
                                :, 128 * g + 32 * c : 128 * g + 32 * c + 32
                            ],
                            rhs=zp[:, c * bl : (c + 1) * bl],
                            start=True,
                            stop=True,
                            tile_position=(0, 32 * c),
                        )

                    nout = 16 * npair  # output partitions (112 for L1 g31)
                    if layer < 2:
                        hb = opool.tile([128, bl], BF16, tag="hb")
                        nc.scalar.copy(out=hb[0:nout, :], in_=psum[0:nout, :])
                        mw = nc.gpsimd.dma_start(
                            out=mirrors[layer + 1][
                                128 * g : 128 * g + nout, :
                            ],
                            in_=hb[0:nout, :],
                        )
                        writers[(layer + 1, g)] = [mw]
                    else:
                        osb = opool.tile([128, bl], F32, tag="osb")
                        nc.scalar.activation(
                            osb[:],
                            psum[:],
                            mybir.ActivationFunctionType.Identity,
                            bias=bib[:, g : g + 1],
                            scale=scb[:, g : g + 1],
                        )
                        nc.gpsimd.dma_start(out=out_d[g], in_=osb[:])
                if layer == 1:
                    # passthrough sites into the s-layout mirror tail
                    p_a = nc.sync.dma_start(
                        out=hm2[31 * 128 + 112 : 31 * 128 + 120, :],
                        in_=hm1[0:8, :],
                    )
                    for w in writers.get((1, 0), ()):
                        _dep(p_a, w, "site0 passthrough after hm1 blk0")
                    p_b = nc.sync.dma_start(
                        out=hm2[31 * 128 + 120 : 31 * 128 + 128, :],
                        in_=hm1[F - 8 : F, :],
                    )
                    for w in writers.get((1, 31), ()):
                        _dep(p_b, w, "site511 passthrough after hm1 blk31")
                    writers[(2, 31)] = writers.get((2, 31), []) + [p_a, p_b]

    nc.finalize()
    return nc


# ---------------------------------------------------------------------------
# host-side prep
# ---------------------------------------------------------------------------
def _build_l0_slabs(xc):
    """xc: (bl, 4096) permuted core shard -> (z1, z2) each (32, 128, 4*bl)."""
    bl = xc.shape[0]
    xs = np.ascontiguousarray(xc.T.reshape(NS, D, bl))  # (site, d, b)
    # sites s1 = 16g+4c+2a, s2 = s1+1
    g_, c_, a_ = np.meshgrid(
        np.arange(32), np.arange(4), np.arange(2), indexing="ij"
    )
    s1 = 16 * g_ + 4 * c_ + 2 * a_  # (32, 4, 2)
    x1 = xs[s1]  # (32, 4, 2, 8, bl)
    x2 = xs[s1 + 1]
    # z1[g, p=(a,i,j), c, b] = x1[g, c, a, i, b]
    z1 = np.broadcast_to(
        x1[:, :, :, :, None, :], (32, 4, 2, 8, 8, bl)
    )  # (g, c, a, i, j, b)
    z1 = np.ascontiguousarray(z1.transpose(0, 2, 3, 4, 1, 5)).reshape(
        32, 128, 4 * bl
    )
    z2 = np.broadcast_to(x2[:, :, :, None, :, :], (32, 4, 2, 8, 8, bl))
    z2 = np.ascontiguousarray(z2.transpose(0, 2, 3, 4, 1, 5)).reshape(
        32, 128, 4 * bl
    )
    return z1, z2


def _prep_host(x, gates0, gates1, gates2, alpha, per_dim_scale, bias, input_perm):
    Ws = [_fold_gates(np.asarray(g)) for g in (gates0, gates1, gates2)]
    wbs = [_wbuf_for_layer(Ws[l], l) for l in range(3)]
    x_perm = np.asarray(x)[:, np.asarray(input_perm)]
    sc = (
        np.float32(np.asarray(alpha).reshape(-1)[0]) * np.asarray(per_dim_scale)
    ).astype(np.float32)
    bi = np.asarray(bias).astype(np.float32)
    scb = np.ascontiguousarray(sc.reshape(NBLK, 128).T)  # (128, NBLK)
    bib = np.ascontiguousarray(bi.reshape(NBLK, 128).T)
    return Ws, wbs, x_perm, scb, bib


def make_in_maps(x_perm, wbs, scb, bib, bl=BL, ncores=NCORES):
    import ml_dtypes

    bf16 = ml_dtypes.bfloat16
    w_bf = [w.astype(bf16) for w in wbs]
    in_maps = []
    for cid in range(ncores):
        xc = x_perm[cid * bl : (cid + 1) * bl].astype(bf16)
        z1, z2 = _build_l0_slabs(xc)
        in_maps.append(
            {
                "z1l0": np.ascontiguousarray(z1),
                "z2l0": np.ascontiguousarray(z2),
                "w0": w_bf[0],
                "w1": w_bf[1],
                "w2": w_bf[2],
                "scb": scb,
                "bib": bib,
            }
        )
    return in_maps


def kernel(x, gates0, gates1, gates2, alpha, per_dim_scale, bias, input_perm):
    Ws, wbs, x_perm, scb, bib = _prep_host(
        x, gates0, gates1, gates2, alpha, per_dim_scale, bias, input_perm
    )
    nc = build_bass(BL)
    in_maps = make_in_maps(x_perm, wbs, scb, bib)
    res = run_bass_kernel_spmd(nc, in_maps, core_ids=list(range(NCORES)))
    outs = res.results if hasattr(res, "results") else res
    full = np.empty((BATCH, F), np.float32)
    for cid in range(NCORES):
        o = np.asarray(outs[cid]["out"]).reshape(NBLK, 128, BL)
        full[cid * BL : (cid + 1) * BL] = o.transpose(2, 0, 1).reshape(BL, F)
    return full



# revision 2
# speedup vs baseline: 8.8658x; 8.8658x over previous
"""Trainium2 Bass kernel for nn_ADTNLinear (3-layer pairwise tensor-network).

Strategy: data-parallel over batch (8 cores x 512 rows). Host folds the rank
axes of the gates into per-pair 64x16 weight blocks, applies the input
permutation, and precomputes layer-0's outer-product operand slabs. On
device, each layer processes 32 groups of 8 site-pairs:
  - operand slabs (x1 replicated over j / x2 tiled over i, [128, 2048] bf16)
    are DMA-gathered from an HBM mirror of the layer input (layer 0: loaded
    directly from the host-built slabs),
  - one DVE tensor_mul forms z = x1rep * x2rep,
  - 16 tile_position-packed 32x32 TensorE matmuls contract z against the
    folded weights into PSUM ([128, 512] f32),
  - ScalarE copies PSUM to an SBUF bounce (bf16) which DMAs to the next
    layer's HBM mirror; layer 2 fuses the alpha*scale+bias affine and DMAs
    the f32 result out.
Layer 1 (offset=1, 255 pairs) writes a shifted "s-layout" mirror so PSUM
writebacks stay partition-aligned; layer 2's gather offsets absorb the shift.
"""

import sys

sys.path.insert(0, "/opt/trn_rl_repo")

import numpy as np

import concourse.bacc as bacc
import concourse.bass as bass
import concourse.mybir as mybir
from concourse import tile
from concourse.bass import _add_dep_helper
from concourse.bass_utils import run_bass_kernel_spmd


def _minst(ret):
    return ret.ins if hasattr(ret, "ins") else ret


def _dep(after, before, reason):
    """after must run after before (raw-AP reads are invisible to Tile)."""
    if before is None:
        return
    _add_dep_helper(_minst(after), _minst(before), sync=True, reason=reason)

NCORES = 8
BATCH = 4096
BL = BATCH // NCORES  # 512 batch rows per core
NS = 512  # sites
D = 8  # features per site
F = NS * D  # 4096
NBLK = F // 128  # 32 feature blocks
BF16 = mybir.dt.bfloat16
F32 = mybir.dt.float32


def _fold_gates(g):
    """gates (P, rl, rr, i, j, o, p) -> W (P, 64, 16) with z-row = 8*i + j."""
    G = g.sum(axis=(1, 2))  # (P, i, j, o, p)
    w1 = G.sum(-1)  # (P, i, j, m)  site1 outputs
    w2 = G.sum(-2)  # (P, i, j, n)  site2 outputs
    P = G.shape[0]
    W = np.concatenate([w1, w2], axis=-1)  # (P, i, j, 16)
    return W.reshape(P, 64, 16).astype(np.float32)


def _layer_pairs(layer):
    return 256 if layer != 1 else 255


def _wbuf_for_layer(W, layer):
    """Host-side SBUF image of the packed weights: (128, 32*128) f32.

    Tile (r, c) of group g holds lhsT[32, 32]: rows = z-rows 32r..32r+32 of
    pair kA=8g+2c (r<2, cols 0:16) or pair kB=8g+2c+1 (r>=2, cols 16:32).
    """
    P = W.shape[0]
    wb = np.zeros((128, NBLK * 128), np.float32)
    for g in range(32):
        for c in range(4):
            kA = 8 * g + 2 * c
            kB = kA + 1
            col = 128 * g + 32 * c
            if kA < P:
                wb[0:64, col : col + 16] = W[kA]
            if kB < P:
                wb[64:128, col + 16 : col + 32] = W[kB]
    return wb


def _src_feat_row(layer, g, c, a, operand):
    """Feature row (0..4095) in the layer's input mirror for one gather.

    The mirror for layer l stores feature f at row: f (canonical, layers
    0/1) or (f-8) % 4096 reordered per the s-layout (layer 2): feats
    8..4088 at rows 0..4080, site0 at rows 3952+? -- concretely the
    s-layout mirror row of feat f is (f-8) mod 128 + 128*((f-8)//128) for
    f in [8, 4088), site0 (f 0..8) at block31 parts 112..120 = rows
    4080..4088, site511 (f 4088..96) at rows 4088..4096.
    """
    o = 1 if layer == 1 else 0
    k = 8 * g + 2 * c + a
    s = 2 * k + o + (1 if operand == 2 else 0)
    f = 8 * s
    if layer in (0, 1):
        return f
    if s == 0:
        return 31 * 128 + 112
    if s == 511:
        return 31 * 128 + 120
    return f - 8


def build_bass(bl=BL):
    """Build the single-core graph (SPMD: all 8 cores run it on their shard)."""
    nc = bacc.Bacc(None, target_bir_lowering=False, debug=False)

    z1l0 = nc.declare_dram_parameter("z1l0", [32, 128, 4 * bl], BF16, isOutput=False)
    z2l0 = nc.declare_dram_parameter("z2l0", [32, 128, 4 * bl], BF16, isOutput=False)
    wbs = [
        nc.declare_dram_parameter(f"w{l}", [128, NBLK * 128], BF16, isOutput=False)
        for l in range(3)
    ]
    scb_d = nc.declare_dram_parameter("scb", [128, NBLK], F32, isOutput=False)
    bib_d = nc.declare_dram_parameter("bib", [128, NBLK], F32, isOutput=False)
    out_d = nc.declare_dram_parameter("out", [NBLK, 128, bl], F32, isOutput=True)

    with tile.TileContext(nc) as tc:
        with (
            tc.tile_pool(name="wbuf", bufs=1) as wpool,
            tc.tile_pool(name="zbuf", bufs=4) as zpool,
            tc.tile_pool(name="obuf", bufs=4) as opool,
            tc.tile_pool(name="dram", bufs=1, space="DRAM") as dpool,
            tc.tile_pool(name="psum", bufs=6, space="PSUM") as pspool,
        ):
            wt = [
                wpool.tile([128, NBLK * 128], BF16, tag=f"w{l}", name=f"wt{l}")
                for l in range(3)
            ]
            scb = wpool.tile([128, NBLK], F32, tag="scb")
            bib = wpool.tile([128, NBLK], F32, tag="bib")
            hm1 = dpool.tile([F, bl], BF16, tag="hm1")  # canonical mirror
            hm2 = dpool.tile([F, bl], BF16, tag="hm2")  # s-layout mirror

            for l in range(3):
                nc.sync.dma_start(out=wt[l][:], in_=wbs[l][:])
            nc.sync.dma_start(out=scb[:], in_=scb_d[:])
            nc.sync.dma_start(out=bib[:], in_=bib_d[:])

            mirrors = {1: hm1, 2: hm2}
            # writers[(mirror_idx, blk)] = insts that wrote that 128-row block
            writers = {}

            for layer in range(3):
                P = _layer_pairs(layer)
                for g in range(32):
                    npair = min(8, P - 8 * g)  # 8, or 7 for layer-1 group 31
                    zb1 = zpool.tile([128, 4 * bl], BF16, tag="zb1")
                    zb2 = zpool.tile([128, 4 * bl], BF16, tag="zb2")
                    zp = zpool.tile([128, 4 * bl], BF16, tag="zp")
                    if layer == 0:
                        nc.sync.dma_start(out=zb1[:], in_=z1l0[g])
                        nc.scalar.dma_start(out=zb2[:], in_=z2l0[g])
                    else:
                        hm = mirrors[layer]
                        if npair < 8:  # partial group: zero unwritten region
                            nc.vector.memset(zb1[:], 0.0)
                            nc.vector.memset(zb2[:], 0.0)
                        for c in range(4):
                            for a in range(2):
                                if 8 * g + 2 * c + a >= P:
                                    continue
                                f1 = _src_feat_row(layer, g, c, a, 1)
                                # x1 replicated over j: iter (i, j-rep, b)
                                ap1 = bass.AP(
                                    hm[:].tensor,
                                    f1 * bl,
                                    [[bl, 8], [0, 8], [1, bl]],
                                )
                                d1 = nc.sync.dma_start(
                                    out=zb1[
                                        64 * a : 64 * a + 64,
                                        c * bl : (c + 1) * bl,
                                    ],
                                    in_=ap1,
                                )
                                for w in writers.get((layer, f1 // 128), ()):
                                    _dep(d1, w, "x1 gather after mirror write")
                                f2 = _src_feat_row(layer, g, c, a, 2)
                                # x2 tiled over i: iter (i-rep, j, b)
                                ap2 = bass.AP(
                                    hm[:].tensor,
                                    f2 * bl,
                                    [[0, 8], [bl, 8], [1, bl]],
                                )
                                d2 = nc.scalar.dma_start(
                                    out=zb2[
                                        64 * a : 64 * a + 64,
                                        c * bl : (c + 1) * bl,
                                    ],
                                    in_=ap2,
                                )
                                for w in writers.get((layer, f2 // 128), ()):
                                    _dep(d2, w, "x2 gather after mirror write")
                    nc.vector.tensor_mul(zp[:], zb1[:], zb2[:])

                    psum = pspool.tile([128, bl], F32, tag="ps")
                    for c in range(4):
                        if 8 * g + 2 * c >= P:
                            continue
                        # block-diag 2-pair weights: K=128 (2x64 z-rows), M=32;
                        # explicit col-group tile_position -> 4 concurrent MMs
                        nc.tensor.matmul(
                            psum[32 * c : 32 * c + 32, :],
                            lhsT=wt[layer][
                                :, 128 * g + 32 * c : 128 * g + 32 * c + 32
                            ],
                            rhs=zp[:, c * bl : (c + 1) * bl],
                            start=True,
                            stop=True,
                            tile_position=(0, 32 * c),
                        )

                    nout = 16 * npair  # output partitions (112 for L1 g31)
                    if layer < 2:
                        hb = opool.tile([128, bl], BF16, tag="hb")
                        nc.scalar.copy(out=hb[0:nout, :], in_=psum[0:nout, :])
                        mw = nc.gpsimd.dma_start(
                            out=mirrors[layer + 1][
                                128 * g : 128 * g + nout, :
                            ],
                            in_=hb[0:nout, :],
                        )
                        writers[(layer + 1, g)] = [mw]
                    else:
                        osb = opool.tile([128, bl], F32, tag="osb")
                        nc.scalar.activation(
                            osb[:],
                            psum[:],
                            mybir.ActivationFunctionType.Identity,
                            bias=bib[:, g : g + 1],
                            scale=scb[:, g : g + 1],
                        )
                        nc.gpsimd.dma_start(out=out_d[g], in_=osb[:])
                if layer == 1:
                    # passthrough sites into the s-layout mirror tail
                    p_a = nc.sync.dma_start(
                        out=hm2[31 * 128 + 112 : 31 * 128 + 120, :],
                        in_=hm1[0:8, :],
                    )
                    for w in writers.get((1, 0), ()):
                        _dep(p_a, w, "site0 passthrough after hm1 blk0")
                    p_b = nc.sync.dma_start(
                        out=hm2[31 * 128 + 120 : 31 * 128 + 128, :],
                        in_=hm1[F - 8 : F, :],
                    )
                    for w in writers.get((1, 31), ()):
                        _dep(p_b, w, "site511 passthrough after hm1 blk31")
                    writers[(2, 31)] = writers.get((2, 31), []) + [p_a, p_b]

    nc.finalize()
    return nc


# ---------------------------------------------------------------------------
# host-side prep
# ---------------------------------------------------------------------------
def _build_l0_slabs(xc):
    """xc: (bl, 4096) permuted core shard -> (z1, z2) each (32, 128, 4*bl)."""
    bl = xc.shape[0]
    xs = np.ascontiguousarray(xc.T.reshape(NS, D, bl))  # (site, d, b)
    # sites s1 = 16g+4c+2a, s2 = s1+1
    g_, c_, a_ = np.meshgrid(
        np.arange(32), np.arange(4), np.arange(2), indexing="ij"
    )
    s1 = 16 * g_ + 4 * c_ + 2 * a_  # (32, 4, 2)
    x1 = xs[s1]  # (32, 4, 2, 8, bl)
    x2 = xs[s1 + 1]
    # z1[g, p=(a,i,j), c, b] = x1[g, c, a, i, b]
    z1 = np.broadcast_to(
        x1[:, :, :, :, None, :], (32, 4, 2, 8, 8, bl)
    )  # (g, c, a, i, j, b)
    z1 = np.ascontiguousarray(z1.transpose(0, 2, 3, 4, 1, 5)).reshape(
        32, 128, 4 * bl
    )
    z2 = np.broadcast_to(x2[:, :, :, None, :, :], (32, 4, 2, 8, 8, bl))
    z2 = np.ascontiguousarray(z2.transpose(0, 2, 3, 4, 1, 5)).reshape(
        32, 128, 4 * bl
    )
    return z1, z2


def _prep_host(x, gates0, gates1, gates2, alpha, per_dim_scale, bias, input_perm):
    Ws = [_fold_gates(np.asarray(g)) for g in (gates0, gates1, gates2)]
    wbs = [_wbuf_for_layer(Ws[l], l) for l in range(3)]
    x_perm = np.asarray(x)[:, np.asarray(input_perm)]
    sc = (
        np.float32(np.asarray(alpha).reshape(-1)[0]) * np.asarray(per_dim_scale)
    ).astype(np.float32)
    bi = np.asarray(bias).astype(np.float32)
    scb = np.ascontiguousarray(sc.reshape(NBLK, 128).T)  # (128, NBLK)
    bib = np.ascontiguousarray(bi.reshape(NBLK, 128).T)
    return Ws, wbs, x_perm, scb, bib


def make_in_maps(x_perm, wbs, scb, bib, bl=BL, ncores=NCORES):
    import ml_dtypes

    bf16 = ml_dtypes.bfloat16
    w_bf = [w.astype(bf16) for w in wbs]
    in_maps = []
    for cid in range(ncores):
        xc = x_perm[cid * bl : (cid + 1) * bl].astype(bf16)
        z1, z2 = _build_l0_slabs(xc)
        in_maps.append(
            {
                "z1l0": np.ascontiguousarray(z1),
                "z2l0": np.ascontiguousarray(z2),
                "w0": w_bf[0],
                "w1": w_bf[1],
                "w2": w_bf[2],
                "scb": scb,
                "bib": bib,
            }
        )
    return in_maps


def make_in_maps_from_inputs(inputs):
    Ws, wbs, x_perm, scb, bib = _prep_host(**inputs)
    return make_in_maps(x_perm, wbs, scb, bib)


def assemble_output(outs, inputs=None):
    full = np.empty((BATCH, F), np.float32)
    for cid in range(NCORES):
        o = np.asarray(outs[cid]["out"]).reshape(NBLK, 128, BL)
        full[cid * BL : (cid + 1) * BL] = o.transpose(2, 0, 1).reshape(BL, F)
    return full


def kernel(x, gates0, gates1, gates2, alpha, per_dim_scale, bias, input_perm):
    Ws, wbs, x_perm, scb, bib = _prep_host(
        x, gates0, gates1, gates2, alpha, per_dim_scale, bias, input_perm
    )
    nc = build_bass(BL)
    in_maps = make_in_maps(x_perm, wbs, scb, bib)
    res = run_bass_kernel_spmd(nc, in_maps, core_ids=list(range(NCORES)))
    outs = res.results if hasattr(res, "results") else res
    full = np.empty((BATCH, F), np.float32)
    for cid in range(NCORES):
        o = np.asarray(outs[cid]["out"]).reshape(NBLK, 128, BL)
        full[cid * BL : (cid + 1) * BL] = o.transpose(2, 0, 1).reshape(BL, F)
    return full

